# revision 1
# baseline (speedup 1.0000x reference)
"""Multi-head GQA attention (B=4, S=2048, D=4096, H=32, KVH=8, HD=128,
start_pos=0, no mask) on 8 Trainium2 NeuronCores.

Sharding: core c -> batch b = c//2, query-token half hh = c%2 (1024 q
tokens). K/V projection work is additionally split across the core
pair (each core projects K/V only for its own 1024 tokens) and the
halves are exchanged with in-pair AllGathers that overlap the Q
projection, so no projection FLOP is duplicated.

Design notes:
 - host pre-transposes x to x^T tiles (bf16) -> zero on-device x
   transposes; host pre-tiles all weights (bf16) into the exact
   stationary layout -> each weight byte is streamed from HBM once.
 - K^T and V (natural layout) are SBUF-resident; no DRAM spills.
 - all matmuls bf16 with fp32 PSUM accumulation (~120 GFLOP/core).
 - attention phase is Activation(exp)-throughput-bound; score tiles
   are paired so each exp covers 1024 columns, and the Q projection
   for the second token-quarter is interleaved into the attention
   loop as PE filler, one (head,kv-pair) stage ahead of the PV
   matmuls -> the merged phase is PE-bound.
 - output projection y^T accumulated over heads in PSUM, evicted via
   alternating PSUM pools with double-buffered staging.
 - x^T tiles load as fine-grained per-2-chunk DMAs (dependency
   tracking is sub-AP granular, so consumers start on the first chunk
   and buffer-reuse hazards resolve chunk by chunk); post-AllGather
   K/V readbacks are chunked per head-group so weight streams
   interleave with them.

RoPE trick: host pre-permutes wq/wk columns per head into the
"evens||odds" basis so the interleaved complex rotation becomes two
contiguous 64-partition halves; q.k dots are invariant to the shared
permutation and V/wo are untouched.

Softmax: no max-subtraction (|scores|*scale stays well inside fp32 exp
range). Denominator via DVE add-chain over exp tiles + one ones-matmul
to broadcast the partition-sum.
"""
import numpy as np
from contextlib import ExitStack

B, S, D, H, KVH, HD = 4, 2048, 4096, 32, 8, 128
NCORES = 8
TQ = S // 2          # q tokens per core
CC = D // 128        # 32 contraction chunks
NQ4 = S // 512       # 4 token quarters
KC = S // 128        # 16 kv chunks
SCALE = 1.0 / float(np.sqrt(HD))

_prog = None
last_exec_ns = None


def _build_program():
    import concourse.tile as tile
    from concourse import bacc, mybir
    from concourse.masks import make_identity

    f32 = mybir.dt.float32
    bf16 = mybir.dt.bfloat16
    EXP = mybir.ActivationFunctionType.Exp

    nc = bacc.Bacc("TRN2", target_bir_lowering=False, debug=False)
    # x^T pretiled, OWN token half only: [q, p, cc, t]
    xt = nc.dram_tensor("xt", [2, 128, CC, 512], bf16, kind="ExternalInput")
    # weights pretiled as stationary chunks (bf16, rope-permuted q/k cols)
    wqt = nc.dram_tensor("wqt", [H, 128, CC, 128], bf16, kind="ExternalInput")
    wkt = nc.dram_tensor("wkt", [KVH, 128, CC, 128], bf16, kind="ExternalInput")
    wvt = nc.dram_tensor("wvt", [KVH, 128, CC, 128], bf16, kind="ExternalInput")
    # wo pretiled: [dm, p, h, e] = wo[h*128+p, dm*128+e]
    wot = nc.dram_tensor("wot", [CC, 128, H, 128], bf16, kind="ExternalInput")
    # cos/sin packed: [quarter, p, t]; rows 0:64 cos^T, 64:128 sin^T
    cst = nc.dram_tensor("cst", [2, 128, 512], f32, kind="ExternalInput")
    yT = nc.dram_tensor("yT", [D, TQ], f32, kind="ExternalOutput")

    with tile.TileContext(nc) as tc, ExitStack() as ctx:
        consts = ctx.enter_context(tc.tile_pool(name="consts", bufs=1))
        dram = ctx.enter_context(tc.tile_pool(name="dram", bufs=1, space="DRAM"))
        xtp = ctx.enter_context(tc.tile_pool(name="xtp", bufs=1))
        qa_p = ctx.enter_context(tc.tile_pool(name="qa", bufs=1))
        kt_p = ctx.enter_context(tc.tile_pool(name="kt", bufs=1, side="right"))
        vn_p = ctx.enter_context(tc.tile_pool(name="vn", bufs=1, side="right"))
        wstr = ctx.enter_context(tc.tile_pool(name="wstr", bufs=2))
        cs_p = ctx.enter_context(tc.tile_pool(name="cs", bufs=1))
        vst = ctx.enter_context(tc.tile_pool(name="vst", bufs=2))
        expp = ctx.enter_context(tc.tile_pool(name="expp", bufs=5))
        dacc = ctx.enter_context(tc.tile_pool(name="dacc", bufs=3))
        small = ctx.enter_context(tc.tile_pool(name="small", bufs=1))
        ostg = ctx.enter_context(tc.tile_pool(name="ostg", bufs=2))

        psA = ctx.enter_context(tc.tile_pool(name="psA", bufs=2, space="PSUM"))
        psS = ctx.enter_context(tc.tile_pool(name="psS", bufs=2, space="PSUM"))
        psB = ctx.enter_context(tc.tile_pool(name="psB", bufs=2, space="PSUM"))

        ident_bf = consts.tile([128, 128], bf16)
        make_identity(nc, ident_bf)
        ones = consts.tile([128, 128], bf16)
        nc.vector.memset(ones, 1.0)

        # resident K^T [hd, g, kv] and V natural [kv%128, g, kvchunk, hd]
        KT = kt_p.tile([128, KVH, S], bf16, tag="KT")
        Vn = vn_p.tile([128, KVH, KC, 128], bf16, tag="Vn")
        # Q for all heads over own 1024 tokens; attention output overwrites
        QA = qa_p.tile([128, H, TQ], bf16, tag="QA")

        def rope(src, cs, dst):
            lo, hi = src[0:64, :], src[64:128, :]
            c, s = cs[0:64, :], cs[64:128, :]
            t1 = small.tile([64, 512], bf16, tag="r1")
            t2 = small.tile([64, 512], bf16, tag="r2")
            nc.vector.tensor_mul(t1, lo, c)
            nc.vector.tensor_mul(t2, hi, s)
            nc.vector.tensor_sub(dst[0:64, :], t1, t2)
            t3 = small.tile([64, 512], bf16, tag="r1")
            t4 = small.tile([64, 512], bf16, tag="r2")
            nc.vector.tensor_mul(t3, lo, s)
            nc.vector.tensor_mul(t4, hi, c)
            nc.vector.tensor_add(dst[64:128, :], t3, t4)

        # ===== Phase A: split K/V across the core pair + AllGather =====
        # Each core projects K/V only for its OWN 1024 tokens, stages them
        # to DRAM, and an in-pair AllGather + readback assembles the full
        # 2048-token K^T/V while the Q projection keeps the PE busy.
        k_own = dram.tile([128, KVH, TQ], bf16, tag="kown")
        k_all = dram.tile([2, 128, KVH, TQ], bf16, tag="kall")
        v_own = dram.tile([128, KVH, TQ], bf16, tag="vown")
        v_all = dram.tile([2, 128, KVH, TQ], bf16, tag="vall")

        def flush_vt(g, q4, raw):
            # transposes for V head g, deferred one head so the PSUM->SBUF
            # copy latency hides behind the next projection chain
            tp = psA.tile([128, 4, 128], bf16, tag="pa")
            for j in range(4):
                nc.tensor.transpose(tp[:, j, :],
                                    raw[:, j * 128:(j + 1) * 128], ident_bf)
            vs = vst.tile([128, 4, 128], bf16, tag="vstg", bufs=1)
            nc.scalar.copy(vs, tp)
            nc.sync.dma_start(
                out=v_own[:, g, q4 * 512:(q4 + 1) * 512]
                .rearrange("p (k e) -> p k e", k=4),
                in_=vs)

        def load_xq(q4, name, queues=None):
            # fine-grained per-2-chunk loads: the dep tracker is sub-AP
            # granular, so consumers start as soon as their chunks land
            # and buffer-reuse WARs resolve chunk by chunk.
            xq = xtp.tile([128, CC, 512], bf16, tag="xq", name=name)
            if queues is None:
                queues = (nc.sync, nc.scalar)
            for i in range(CC // 2):
                q = queues[i % len(queues)]
                q.dma_start(out=xq[:, 2 * i:2 * i + 2, :],
                            in_=xt.ap()[q4][:, 2 * i:2 * i + 2, :])
            return xq

        wt0 = wstr.tile([128, CC, 128], bf16, tag="wt", name="wt0")
        nc.scalar.dma_start(out=wt0[:, 0:CC // 2, :],
                            in_=wkt.ap()[0][:, 0:CC // 2, :])
        nc.scalar.dma_start(out=wt0[:, CC // 2:CC, :],
                            in_=wkt.ap()[0][:, CC // 2:CC, :])
        for q4 in (0, 1):                   # own quarters: K/V projection
            xq = load_xq(q4, f"xqa{q4}", queues=(nc.sync, nc.gpsimd))
            cs = cs_p.tile([128, 512], f32, tag="cs", name=f"csk{q4}")
            nc.gpsimd.dma_start(out=cs, in_=cst.ap()[q4])
            for g in range(KVH):            # K heads
                if q4 == 0 and g == 0:
                    wt = wt0
                else:
                    wt = wstr.tile([128, CC, 128], bf16, tag="wt")
                    nc.sync.dma_start(out=wt, in_=wkt.ap()[g])
                pp = psA.tile([128, 512], f32, tag="pa")
                for cc in range(CC):
                    nc.tensor.matmul(pp, wt[:, cc, :], xq[:, cc, :],
                                     start=(cc == 0), stop=(cc == CC - 1))
                kst = vst.tile([128, 512], bf16, tag="kst", bufs=1)
                rope(pp, cs, kst)
                nc.sync.dma_start(
                    out=k_own[:, g, q4 * 512:(q4 + 1) * 512],
                    in_=kst)

            pend_v = None
            for g in range(KVH):            # V heads
                wt = wstr.tile([128, CC, 128], bf16, tag="wt")
                nc.sync.dma_start(out=wt, in_=wvt.ap()[g])
                pp = psA.tile([128, 512], f32, tag="pa")
                for cc in range(CC):
                    nc.tensor.matmul(pp, wt[:, cc, :], xq[:, cc, :],
                                     start=(cc == 0), stop=(cc == CC - 1))
                raw = vst.tile([128, 512], bf16, tag="raw")
                nc.scalar.copy(raw, pp)
                if pend_v is not None:
                    flush_vt(pend_v[0], q4, pend_v[1])
                pend_v = (g, raw)
            flush_vt(pend_v[0], q4, pend_v[1])

        # pair AllGathers of K then V (gpsimd queue; overlap Q projection)
        nc.gpsimd.collective_compute(
            "AllGather", mybir.AluOpType.bypass,
            replica_groups=[[0, 1], [2, 3], [4, 5], [6, 7]],
            ins=[k_own.opt()], outs=[k_all.opt()])
        nc.gpsimd.collective_compute(
            "AllGather", mybir.AluOpType.bypass,
            replica_groups=[[0, 1], [2, 3], [4, 5], [6, 7]],
            ins=[v_own.opt()], outs=[v_all.opt()])
        for g4 in range(8):         # head-group 0 first: B(h=0) needs it
            for rank in range(2):
                nc.gpsimd.dma_start(
                    out=KT[:, g4:g4 + 1, rank * TQ:(rank + 1) * TQ],
                    in_=k_all[rank][:, g4:g4 + 1, :])
        for g4 in range(8):
            for rank in range(2):
                nc.gpsimd.dma_start(
                    out=Vn[:, g4:g4 + 1, rank * 8:(rank + 1) * 8, :],
                    in_=v_all[rank][:, g4:g4 + 1, :]
                    .rearrange("p g (k e) -> p g k e", k=8))

        xq = load_xq(0, "xq0")
        cs = cs_p.tile([128, 512], f32, tag="cs", name="cs0")
        nc.sync.dma_start(out=cs, in_=cst.ap()[0])
        for h in range(H):                   # Q projection, first quarter
            wt = wstr.tile([128, CC, 128], bf16, tag="wt")
            nc.sync.dma_start(out=wt, in_=wqt.ap()[h])
            pq = psA.tile([128, 512], f32, tag="pa")
            for cc in range(CC):
                nc.tensor.matmul(pq, wt[:, cc, :], xq[:, cc, :],
                                 start=(cc == 0), stop=(cc == CC - 1))
            rope(pq, cs, QA[:, h, 0:512])

        # ===== Phase B: attention (attnout overwrites QA per head) =====
        # 1024-wide bf16 exp tiles; PE emission runs one (h,kp) stage ahead
        # of the PV matmuls so the exp latency is fully hidden and the
        # phase is bound by Activation throughput alone.
        NKP = KC // 2
        pvs, accs = {}, {}

        def emit_den_mul(h):
            for qt in range(2):
                fa = dacc.tile([128, 512], bf16, tag="fa", bufs=1)
                nc.vector.tensor_add(fa, accs[h][qt][:, 0, :],
                                     accs[h][qt][:, 1, :])
                den = psA.tile([128, 512], f32, tag="pa")
                nc.tensor.matmul(den, ones, fa)
                rc = ostg.tile([128, 512], f32, tag="rc", bufs=1)
                nc.vector.reciprocal(rc, den)
                nc.vector.tensor_mul(QA[:, h, qt * 512:(qt + 1) * 512],
                                     pvs[h][qt], rc)

        def emit_pv_acc(g, h, kp, pts):
            if kp == 0:
                pvs[h] = [psB.tile([128, 512], f32, tag="pv", name=f"pv{i}")
                          for i in range(2)]
                accs[h] = [None, None]
            for j in range(2):
                kc = 2 * kp + j
                for qt in range(2):
                    nc.tensor.matmul(pvs[h][qt], Vn[:, g, kc, :],
                                     pts[qt][:, j, :],
                                     start=(kc == 0), stop=(kc == KC - 1))
            for qt in range(2):
                if kp == 0:
                    accs[h][qt] = pts[qt]
                else:
                    na = dacc.tile([128, 2, 512], bf16, tag="da")
                    nc.vector.tensor_add(na, accs[h][qt], pts[qt])
                    accs[h][qt] = na

        xq1 = load_xq(1, "xq1")
        cs1 = cs_p.tile([128, 512], f32, tag="cs", name="cs1")
        nc.sync.dma_start(out=cs1, in_=cst.ap()[1])

        # prologue: Q(q4=1) for head 0
        wt = wstr.tile([128, CC, 128], bf16, tag="wt")
        nc.sync.dma_start(out=wt, in_=wqt.ap()[0])
        pq = psA.tile([128, 512], f32, tag="pa")
        for cc in range(CC):
            nc.tensor.matmul(pq, wt[:, cc, :], xq1[:, cc, :],
                             start=(cc == 0), stop=(cc == CC - 1))
        rope(pq, cs1, QA[:, 0, 512:1024])

        # merged loop: attention for head h + Q(q4=1) chain for head h+1
        # spread as PE filler between the Act-bound score/exp groups
        pend = None
        for h in range(H):
            g = h // 4
            if h + 1 < H:
                wt = wstr.tile([128, CC, 128], bf16, tag="wt")
                nc.sync.dma_start(out=wt, in_=wqt.ap()[h + 1])
                pq = psA.tile([128, 512], f32, tag="pa")
            for kp in range(NKP):
                scs = [psS.tile([128, 2, 512], f32, tag="sc", name=f"sc{i}")
                       for i in range(2)]
                for j in range(2):
                    kc = 2 * kp + j
                    for qt in range(2):
                        nc.tensor.matmul(
                            scs[qt][:, j, :],
                            KT[:, g, kc * 128:(kc + 1) * 128],
                            QA[:, h, qt * 512:(qt + 1) * 512])
                pts = []
                for qt in range(2):
                    pt = expp.tile([128, 2, 512], bf16, tag="pt")
                    nc.scalar.activation(pt, scs[qt], EXP, scale=SCALE)
                    pts.append(pt)
                if h + 1 < H:
                    for j in range(4):
                        cc = 4 * kp + j
                        nc.tensor.matmul(pq, wt[:, cc, :], xq1[:, cc, :],
                                         start=(cc == 0), stop=(cc == CC - 1))
                if pend is not None:
                    emit_pv_acc(*pend)
                    if pend[2] == NKP - 1:
                        emit_den_mul(pend[1])
                pend = (g, h, kp, pts)
            if h + 1 < H:
                rope(pq, cs1, QA[:, h + 1, 512:1024])
        emit_pv_acc(*pend)
        emit_den_mul(pend[1])

        # ===== Phase C: output projection y^T = wo^T @ attnout =====
        for dm in range(CC):
            wt = wstr.tile([128, H, 128], bf16, tag="wt")
            nc.sync.dma_start(out=wt, in_=wot.ap()[dm])
            pool = psA if dm % 2 == 0 else psB
            tg = "pa" if dm % 2 == 0 else "pv"
            po = [pool.tile([128, 512], f32, tag=tg, name=f"po{i}")
                  for i in range(2)]
            for h in range(H):
                for qt in range(2):
                    nc.tensor.matmul(po[qt], wt[:, h, :],
                                     QA[:, h, qt * 512:(qt + 1) * 512],
                                     start=(h == 0), stop=(h == H - 1))
            for qt in range(2):
                ot = ostg.tile([128, 512], f32, tag="ot", bufs=2)
                nc.scalar.copy(ot, po[qt])
                nc.sync.dma_start(
                    out=yT.ap()[dm * 128:(dm + 1) * 128,
                                qt * 512:(qt + 1) * 512],
                    in_=ot)
    nc.compile()
    return nc


def _deint_perm():
    return np.arange(HD).reshape(HD // 2, 2).T.reshape(-1).copy()


def kernel(**inputs):
    global _prog, last_exec_ns
    import ml_dtypes
    bf = ml_dtypes.bfloat16
    x = np.asarray(inputs["x"], dtype=np.float32)
    wq = np.asarray(inputs["wq"], dtype=np.float32)
    wk = np.asarray(inputs["wk"], dtype=np.float32)
    wv = np.asarray(inputs["wv"], dtype=np.float32)
    wo = np.asarray(inputs["wo"], dtype=np.float32)
    cos = np.asarray(inputs["cos"], dtype=np.float32)
    sin = np.asarray(inputs["sin"], dtype=np.float32)

    from concourse.bass_utils import run_bass_kernel_spmd

    if _prog is None:
        _prog = _build_program()

    p = _deint_perm()
    permq = np.concatenate([h * HD + p for h in range(H)])
    permk = np.concatenate([g * HD + p for g in range(KVH)])
    # stationary tiling: [out_block, p, cc, e] = w[cc*128+p, ob*128+e]
    def tile_w(w, nb):
        return np.ascontiguousarray(
            w.reshape(CC, 128, nb, 128).transpose(2, 1, 0, 3).astype(bf))
    wqt = tile_w(wq[:, permq], H)
    wkt = tile_w(wk[:, permk], KVH)
    wvt = tile_w(wv, KVH)
    # wo: [dm, p, h, e] = wo[h*128+p, dm*128+e]
    wot = np.ascontiguousarray(
        wo.reshape(H, 128, CC, 128).transpose(2, 1, 0, 3).astype(bf))
    csfull = np.concatenate([cos.T, sin.T], axis=0).astype(np.float32)  # [128,S]

    in_maps = []
    for c in range(NCORES):
        b, hh = c // 2, c % 2
        own = np.arange(hh * TQ, (hh + 1) * TQ)
        xb = x[b].T[:, own].astype(bf)                       # [D, TQ]
        xt = np.ascontiguousarray(
            xb.reshape(CC, 128, 2, 512).transpose(2, 1, 0, 3))
        cst = np.ascontiguousarray(
            csfull[:, own].reshape(128, 2, 512).transpose(1, 0, 2))
        in_maps.append({
            "xt": xt, "wqt": wqt, "wkt": wkt, "wvt": wvt, "wot": wot,
            "cst": cst,
        })

    import os
    trace = bool(os.environ.get("KERNEL_TRACE"))
    res = run_bass_kernel_spmd(_prog, in_maps, core_ids=list(range(NCORES)),
                               trace=trace)
    last_exec_ns = res.exec_time_ns
    out = np.empty((B, S, D), dtype=np.float32)
    for c in range(NCORES):
        b, hh = c // 2, c % 2
        out[b, hh * TQ:(hh + 1) * TQ, :] = res.results[c]["yT"].T
    return out



# revision 4
# speedup vs baseline: 1.1602x; 1.1602x over previous
"""Multi-head GQA attention (B=4, S=2048, D=4096, H=32, KVH=8, HD=128,
start_pos=0, no mask) on 8 Trainium2 NeuronCores.

Sharding: core c -> batch b = c//2, query-token half hh = c%2 (1024 q
tokens). K/V projection work is additionally split across the core
pair (each core projects K/V only for its own 1024 tokens) and the
halves are exchanged with in-pair AllGathers that overlap the Q
projection, so no projection FLOP is duplicated.

Projections (Q/K/V/O) run as residual-fp8 DoubleRow matmuls: operands
are split host-side (or on-device for the attention output) into an
fp8(e4m3) hi part plus an fp8 lo remainder, and each logical matmul
becomes three fp8 DoubleRow matmuls (hi*hi + hi*lo + lo*hi) over
256-deep contraction pairs.  DoubleRow costs 0.5 PE cycles per output
row, so a projection runs at 4/3 the bf16 rate with ~bf16 accuracy
(the dropped lo*lo term is ~0.1%).  Weights carry a x16 scale so their
fp8 lo remainders stay in the normal range; the scale is divided back
out of the exp argument (q.k carries x256) and the final output copy.
The attention core (scores, exp, PV) stays bf16: its contraction depth
(128) only admits a 2x DoubleRow split, which a hi+lo pair cancels out.

Design notes:
 - host pre-transposes x to x^T tiles (fp8 hi+lo); host pre-tiles all
   weights (fp8 hi+lo, x16) into the exact stationary layout.
 - K^T and V (natural layout) are SBUF-resident bf16; no DRAM spills.
 - attention phase: score tiles paired so each exp covers 1024
   columns; the Q projection for the second token-quarter is
   interleaved into the attention loop as PE filler, one (head,
   kv-pair) stage ahead of the PV matmuls.
 - attention output overwrites the Q buffer per head through an fp8
   bitcast view: hi in the first TQ fp8 columns, lo in the second, so
   the O projection reads DoubleRow head-pairs straight out of it.
 - output projection y^T accumulated over heads in PSUM (3 residual
   chains), evicted via alternating PSUM pools, scaled back by 1/256.

RoPE trick: host pre-permutes wq/wk columns per head into the
"evens||odds" basis so the interleaved complex rotation becomes two
contiguous 64-partition halves; q.k dots are invariant to the shared
permutation and V/wo are untouched.

Softmax: no max-subtraction (|scores|*scale stays well inside fp32 exp
range). Denominator via DVE add-chain over exp tiles + one ones-matmul
to broadcast the partition-sum.
"""
import numpy as np
from contextlib import ExitStack

B, S, D, H, KVH, HD = 4, 2048, 4096, 32, 8, 128
NCORES = 8
TQ = S // 2          # q tokens per core
CC = D // 128        # 32 contraction chunks
NCP = CC // 2        # 16 contraction chunk-pairs (DoubleRow)
KC = S // 128        # 16 kv chunks
SCALE = 1.0 / float(np.sqrt(HD))
WS = 16.0            # weight scale: q/k/v/AO carry x16, q.k x256
ISCALE = SCALE / (WS * WS)

_prog = None
last_exec_ns = None


def _build_program():
    import concourse.tile as tile
    from concourse import bacc, mybir
    from concourse.masks import make_identity

    f32 = mybir.dt.float32
    bf16 = mybir.dt.bfloat16
    f8 = mybir.dt.float8e4
    EXP = mybir.ActivationFunctionType.Exp
    DR = mybir.MatmulPerfMode.DoubleRow

    nc = bacc.Bacc("TRN2", target_bir_lowering=False, debug=False)
    # x^T pretiled fp8 hi/lo, OWN token half only: [q4, p, var, cc, t]
    xt = nc.dram_tensor("xt", [2, 128, 2, CC, 512], f8, kind="ExternalInput")
    # weights pretiled fp8 hi/lo as stationary chunks (x16, rope-perm q/k)
    wqt = nc.dram_tensor("wqt", [H, 128, 2, CC, 128], f8, kind="ExternalInput")
    wkt = nc.dram_tensor("wkt", [KVH, 128, 2, CC, 128], f8, kind="ExternalInput")
    wvt = nc.dram_tensor("wvt", [KVH, 128, 2, CC, 128], f8, kind="ExternalInput")
    # wo pretiled: [dm, p, var, h, e] = 16*wo[h*128+p, dm*128+e] hi/lo
    wot = nc.dram_tensor("wot", [CC, 128, 2, H, 128], f8, kind="ExternalInput")
    # cos/sin packed: [quarter, p, t]; rows 0:64 cos^T, 64:128 sin^T
    cst = nc.dram_tensor("cst", [2, 128, 512], f32, kind="ExternalInput")
    yT = nc.dram_tensor("yT", [D, TQ], f32, kind="ExternalOutput")

    # residual-fp8 term order per chunk-pair: (w_var, x_var)
    TERMS = ((0, 0), (0, 1), (1, 0))

    with tile.TileContext(nc) as tc, ExitStack() as ctx:
        consts = ctx.enter_context(tc.tile_pool(name="consts", bufs=1))
        dram = ctx.enter_context(tc.tile_pool(name="dram", bufs=1, space="DRAM"))
        xtp = ctx.enter_context(tc.tile_pool(name="xtp", bufs=1))
        qa_p = ctx.enter_context(tc.tile_pool(name="qa", bufs=1))
        kt_p = ctx.enter_context(tc.tile_pool(name="kt", bufs=1, side="right"))
        vn_p = ctx.enter_context(tc.tile_pool(name="vn", bufs=1, side="right"))
        wstr = ctx.enter_context(tc.tile_pool(name="wstr", bufs=2))
        cs_p = ctx.enter_context(tc.tile_pool(name="cs", bufs=1))
        vst = ctx.enter_context(tc.tile_pool(name="vst", bufs=2))
        expp = ctx.enter_context(tc.tile_pool(name="expp", bufs=4))
        dacc = ctx.enter_context(tc.tile_pool(name="dacc", bufs=3))
        small = ctx.enter_context(tc.tile_pool(name="small", bufs=1))
        ostg = ctx.enter_context(tc.tile_pool(name="ostg", bufs=2))
        aof_p = ctx.enter_context(tc.tile_pool(name="aof", bufs=1))

        psA = ctx.enter_context(tc.tile_pool(name="psA", bufs=2, space="PSUM"))
        psS = ctx.enter_context(tc.tile_pool(name="psS", bufs=2, space="PSUM"))
        psB = ctx.enter_context(tc.tile_pool(name="psB", bufs=2, space="PSUM"))

        ident_bf = consts.tile([128, 128], bf16)
        make_identity(nc, ident_bf)
        ones = consts.tile([128, 128], bf16)
        nc.vector.memset(ones, 1.0)

        # resident K^T [hd, g, kv] and V natural [kv%128, g, kvchunk, hd]
        KT = kt_p.tile([128, KVH, S], bf16, tag="KT")
        Vn = vn_p.tile([128, KVH, KC, 128], bf16, tag="Vn")
        # Q for all heads over own 1024 tokens, as fp8 bytes; rope writes
        # through a bf16 view, attention output (hi|lo fp8) overwrites.
        QA8 = qa_p.tile([128, H, 2 * TQ], f8, tag="QA")
        QA = QA8[:].bitcast(bf16)            # [128, H, TQ] bf16 view

        def rope(src, cs, dst):
            lo, hi = src[0:64, :], src[64:128, :]
            c, s = cs[0:64, :], cs[64:128, :]
            t1 = small.tile([64, 512], bf16, tag="r1")
            t2 = small.tile([64, 512], bf16, tag="r2")
            nc.vector.tensor_mul(t1, lo, c)
            nc.vector.tensor_mul(t2, hi, s)
            nc.vector.tensor_sub(dst[0:64, :], t1, t2)
            t3 = small.tile([64, 512], bf16, tag="r1")
            t4 = small.tile([64, 512], bf16, tag="r2")
            nc.vector.tensor_mul(t3, lo, s)
            nc.vector.tensor_mul(t4, hi, c)
            nc.vector.tensor_add(dst[64:128, :], t3, t4)

        def proj_chain(pp, wt, xq, idx, n=1):
            # residual-fp8 projection: 3 DoubleRow terms x NCP chunk-pairs,
            # emitted chunk-pair-major so x chunks are consumed as they
            # land; idx..idx+n-1 of the 3*NCP schedule.
            for t in range(idx, idx + n):
                cp, trm = divmod(t, len(TERMS))
                wv_, xv_ = TERMS[trm]
                nc.tensor.matmul(
                    pp, wt[:, wv_, 2 * cp:2 * cp + 2, :],
                    xq[:, xv_, 2 * cp:2 * cp + 2, :],
                    start=(t == 0), stop=(t == 3 * NCP - 1), perf_mode=DR)

        # ===== Phase A: split K/V across the core pair + AllGather =====
        k_own = dram.tile([128, KVH, TQ], bf16, tag="kown")
        k_all = dram.tile([2, 128, KVH, TQ], bf16, tag="kall")
        v_own = dram.tile([128, KVH, TQ], bf16, tag="vown")
        v_all = dram.tile([2, 128, KVH, TQ], bf16, tag="vall")

        def flush_vt(g, q4, raw):
            # transposes for V head g, deferred one head so the PSUM->SBUF
            # copy latency hides behind the next projection chain
            tp = psA.tile([128, 4, 128], bf16, tag="pa")
            for j in range(4):
                nc.tensor.transpose(tp[:, j, :],
                                    raw[:, j * 128:(j + 1) * 128], ident_bf)
            vs = vst.tile([128, 4, 128], bf16, tag="vstg", bufs=1)
            nc.scalar.copy(vs, tp)
            nc.sync.dma_start(
                out=v_own[:, g, q4 * 512:(q4 + 1) * 512]
                .rearrange("p (k e) -> p k e", k=4),
                in_=vs)

        def load_xq(q4, name, queues=None):
            # fine-grained per-chunk-pair loads (both fp8 variants): the dep
            # tracker is sub-AP granular, so consumers start as chunks land.
            xq = xtp.tile([128, 2, CC, 512], f8, tag="xq", name=name)
            if queues is None:
                queues = (nc.sync, nc.scalar)
            for i in range(NCP):
                q = queues[i % len(queues)]
                q.dma_start(out=xq[:, :, 2 * i:2 * i + 2, :],
                            in_=xt.ap()[q4][:, :, 2 * i:2 * i + 2, :])
            return xq

        wt0 = wstr.tile([128, 2, CC, 128], f8, tag="wt", name="wt0")
        nc.scalar.dma_start(out=wt0[:, :, 0:CC // 2, :],
                            in_=wkt.ap()[0][:, :, 0:CC // 2, :])
        nc.scalar.dma_start(out=wt0[:, :, CC // 2:CC, :],
                            in_=wkt.ap()[0][:, :, CC // 2:CC, :])
        for q4 in (0, 1):                   # own quarters: K/V projection
            xq = load_xq(q4, f"xqa{q4}", queues=(nc.sync, nc.gpsimd))
            cs = cs_p.tile([128, 512], f32, tag="cs", name=f"csk{q4}")
            nc.gpsimd.dma_start(out=cs, in_=cst.ap()[q4])
            for g in range(KVH):            # K heads
                if q4 == 0 and g == 0:
                    wt = wt0
                else:
                    wt = wstr.tile([128, 2, CC, 128], f8, tag="wt")
                    nc.sync.dma_start(out=wt, in_=wkt.ap()[g])
                pp = psA.tile([128, 512], f32, tag="pa")
                proj_chain(pp, wt, xq, 0, 3 * NCP)
                kst = vst.tile([128, 512], bf16, tag="kst", bufs=1)
                rope(pp, cs, kst)
                nc.sync.dma_start(
                    out=k_own[:, g, q4 * 512:(q4 + 1) * 512],
                    in_=kst)

            pend_v = None
            for g in range(KVH):            # V heads
                wt = wstr.tile([128, 2, CC, 128], f8, tag="wt")
                nc.sync.dma_start(out=wt, in_=wvt.ap()[g])
                pp = psA.tile([128, 512], f32, tag="pa")
                proj_chain(pp, wt, xq, 0, 3 * NCP)
                raw = vst.tile([128, 512], bf16, tag="raw")
                nc.scalar.copy(raw, pp)
                if pend_v is not None:
                    flush_vt(pend_v[0], q4, pend_v[1])
                pend_v = (g, raw)
            flush_vt(pend_v[0], q4, pend_v[1])

        # pair AllGathers of K then V (gpsimd queue; overlap Q projection)
        nc.gpsimd.collective_compute(
            "AllGather", mybir.AluOpType.bypass,
            replica_groups=[[0, 1], [2, 3], [4, 5], [6, 7]],
            ins=[k_own.opt()], outs=[k_all.opt()])
        nc.gpsimd.collective_compute(
            "AllGather", mybir.AluOpType.bypass,
            replica_groups=[[0, 1], [2, 3], [4, 5], [6, 7]],
            ins=[v_own.opt()], outs=[v_all.opt()])
        for g4 in range(8):         # head-group 0 first: B(h=0) needs it
            for rank in range(2):
                nc.gpsimd.dma_start(
                    out=KT[:, g4:g4 + 1, rank * TQ:(rank + 1) * TQ],
                    in_=k_all[rank][:, g4:g4 + 1, :])
        for g4 in range(8):
            for rank in range(2):
                nc.gpsimd.dma_start(
                    out=Vn[:, g4:g4 + 1, rank * 8:(rank + 1) * 8, :],
                    in_=v_all[rank][:, g4:g4 + 1, :]
                    .rearrange("p g (k e) -> p g k e", k=8))

        xq = load_xq(0, "xq0")
        cs = cs_p.tile([128, 512], f32, tag="cs", name="cs0")
        nc.sync.dma_start(out=cs, in_=cst.ap()[0])
        for h in range(H):                   # Q projection, first quarter
            wt = wstr.tile([128, 2, CC, 128], f8, tag="wt")
            nc.sync.dma_start(out=wt, in_=wqt.ap()[h])
            pq = psA.tile([128, 512], f32, tag="pa")
            proj_chain(pq, wt, xq, 0, 3 * NCP)
            rope(pq, cs, QA[:, h, 0:512])

        # ===== Phase B: attention (attnout overwrites QA per head) =====
        # 1024-wide bf16 exp tiles; PE emission runs one (h,kp) stage ahead
        # of the PV matmuls so the exp latency is fully hidden.
        NKP = KC // 2
        pvs, accs = {}, {}

        def emit_den_mul(h):
            for qt in range(2):
                fa = dacc.tile([128, 512], bf16, tag="fa", bufs=1)
                nc.vector.tensor_add(fa, accs[h][qt][:, 0, :],
                                     accs[h][qt][:, 1, :])
                den = psA.tile([128, 512], f32, tag="pa")
                nc.tensor.matmul(den, ones, fa)
                rc = ostg.tile([128, 512], f32, tag="rc", bufs=1)
                nc.vector.reciprocal(rc, den)
                # AO = pvs*rc = 16*attn; store fp8 hi then lo residual,
                # overwriting head h's QA bytes (hi at 0:TQ, lo at TQ:2TQ)
                aof = aof_p.tile([128, 512], f32, tag="aof")
                nc.vector.tensor_mul(aof, pvs[h][qt], rc)
                aoh = QA8[:, h, qt * 512:(qt + 1) * 512]
                aol = QA8[:, h, TQ + qt * 512:TQ + (qt + 1) * 512]
                nc.gpsimd.tensor_copy(aoh, aof)
                nc.gpsimd.tensor_sub(aol, aof, aoh)

        def emit_pv_acc(g, h, kp, pts):
            if kp == 0:
                pvs[h] = [psB.tile([128, 512], f32, tag="pv", name=f"pv{i}")
                          for i in range(2)]
                accs[h] = [None, None]
            for j in range(2):
                kc = 2 * kp + j
                for qt in range(2):
                    nc.tensor.matmul(pvs[h][qt], Vn[:, g, kc, :],
                                     pts[qt][:, j, :],
                                     start=(kc == 0), stop=(kc == KC - 1))
            for qt in range(2):
                if kp == 0:
                    accs[h][qt] = pts[qt]
                else:
                    na = dacc.tile([128, 2, 512], bf16, tag="da")
                    nc.vector.tensor_add(na, accs[h][qt], pts[qt])
                    accs[h][qt] = na

        xq1 = load_xq(1, "xq1")
        cs1 = cs_p.tile([128, 512], f32, tag="cs", name="cs1")
        nc.sync.dma_start(out=cs1, in_=cst.ap()[1])

        # prologue: Q(q4=1) for head 0
        wt = wstr.tile([128, 2, CC, 128], f8, tag="wt")
        nc.sync.dma_start(out=wt, in_=wqt.ap()[0])
        pq = psA.tile([128, 512], f32, tag="pa")
        proj_chain(pq, wt, xq1, 0, 3 * NCP)
        rope(pq, cs1, QA[:, 0, 512:1024])

        # merged loop: attention for head h + Q(q4=1) chain for head h+1
        # spread as PE filler between the Act-bound score/exp groups
        pend = None
        for h in range(H):
            g = h // 4
            if h + 1 < H:
                wt = wstr.tile([128, 2, CC, 128], f8, tag="wt")
                nc.sync.dma_start(out=wt, in_=wqt.ap()[h + 1])
                pq = psA.tile([128, 512], f32, tag="pa")
            for kp in range(NKP):
                scs = [psS.tile([128, 2, 512], f32, tag="sc", name=f"sc{i}")
                       for i in range(2)]
                for j in range(2):
                    kc = 2 * kp + j
                    for qt in range(2):
                        nc.tensor.matmul(
                            scs[qt][:, j, :],
                            KT[:, g, kc * 128:(kc + 1) * 128],
                            QA[:, h, qt * 512:(qt + 1) * 512])
                pts = []
                for qt in range(2):
                    pt = expp.tile([128, 2, 512], bf16, tag="pt")
                    nc.scalar.activation(pt, scs[qt], EXP, scale=ISCALE)
                    pts.append(pt)
                if h + 1 < H:
                    proj_chain(pq, wt, xq1, 6 * kp, 6)
                if pend is not None:
                    emit_pv_acc(*pend)
                    if pend[2] == NKP - 1:
                        emit_den_mul(pend[1])
                pend = (g, h, kp, pts)
            if h + 1 < H:
                rope(pq, cs1, QA[:, h + 1, 512:1024])
        emit_pv_acc(*pend)
        emit_den_mul(pend[1])

        # ===== Phase C: output projection y^T = wo^T @ attnout =====
        # 3 residual DoubleRow chains over 16 head-pairs; AO hi/lo read
        # from the fp8 byte view of QA; result carries x256 -> scale back.
        for dm in range(CC):
            wt = wstr.tile([128, 2, H, 128], f8, tag="wt")
            nc.sync.dma_start(out=wt, in_=wot.ap()[dm])
            pool = psA if dm % 2 == 0 else psB
            tg = "pa" if dm % 2 == 0 else "pv"
            po = [pool.tile([128, 512], f32, tag=tg, name=f"po{i}")
                  for i in range(2)]
            for qt in range(2):
                t = 0
                for wv_, av_ in TERMS:
                    for hp in range(H // 2):
                        nc.tensor.matmul(
                            po[qt], wt[:, wv_, 2 * hp:2 * hp + 2, :],
                            QA8[:, 2 * hp:2 * hp + 2,
                                av_ * TQ + qt * 512:av_ * TQ + (qt + 1) * 512],
                            start=(t == 0), stop=(t == 3 * (H // 2) - 1),
                            perf_mode=DR)
                        t += 1
            for qt in range(2):
                ot = ostg.tile([128, 512], f32, tag="ot", bufs=2)
                nc.scalar.mul(ot, po[qt], 1.0 / (WS * WS))
                nc.sync.dma_start(
                    out=yT.ap()[dm * 128:(dm + 1) * 128,
                                qt * 512:(qt + 1) * 512],
                    in_=ot)
    nc.compile()
    return nc


def _deint_perm():
    return np.arange(HD).reshape(HD // 2, 2).T.reshape(-1).copy()


def _hilo(a):
    import ml_dtypes
    f8 = ml_dtypes.float8_e4m3
    hi = a.astype(f8)
    lo = (a - hi.astype(np.float32)).astype(f8)
    return np.stack([hi, lo])        # [2, ...]


def kernel(**inputs):
    global _prog, last_exec_ns
    x = np.asarray(inputs["x"], dtype=np.float32)
    wq = np.asarray(inputs["wq"], dtype=np.float32)
    wk = np.asarray(inputs["wk"], dtype=np.float32)
    wv = np.asarray(inputs["wv"], dtype=np.float32)
    wo = np.asarray(inputs["wo"], dtype=np.float32)
    cos = np.asarray(inputs["cos"], dtype=np.float32)
    sin = np.asarray(inputs["sin"], dtype=np.float32)

    from concourse.bass_utils import run_bass_kernel_spmd

    if _prog is None:
        _prog = _build_program()

    p = _deint_perm()
    permq = np.concatenate([h * HD + p for h in range(H)])
    permk = np.concatenate([g * HD + p for g in range(KVH)])

    # stationary tiling: [ob, p, var, cc, e] = hilo(16*w)[v, cc*128+p, ob*128+e]
    def tile_w(w, nb):
        hl = _hilo(WS * w)           # [2, D, nb*128]
        return np.ascontiguousarray(
            hl.reshape(2, CC, 128, nb, 128).transpose(3, 2, 0, 1, 4))

    wqt = tile_w(wq[:, permq], H)
    wkt = tile_w(wk[:, permk], KVH)
    wvt = tile_w(wv, KVH)
    # wo: [dm, p, var, h, e] = hilo(16*wo)[v, h*128+p, dm*128+e]
    wol = _hilo(WS * wo)
    wot = np.ascontiguousarray(
        wol.reshape(2, H, 128, CC, 128).transpose(3, 2, 0, 1, 4))
    csfull = np.concatenate([cos.T, sin.T], axis=0).astype(np.float32)  # [128,S]

    in_maps = []
    for c in range(NCORES):
        b, hh = c // 2, c % 2
        own = np.arange(hh * TQ, (hh + 1) * TQ)
        xb = _hilo(x[b].T[:, own])                        # [2, D, TQ]
        xtc = np.ascontiguousarray(
            xb.reshape(2, CC, 128, 2, 512).transpose(3, 2, 0, 1, 4))
        cstc = np.ascontiguousarray(
            csfull[:, own].reshape(128, 2, 512).transpose(1, 0, 2))
        in_maps.append({
            "xt": xtc, "wqt": wqt, "wkt": wkt, "wvt": wvt, "wot": wot,
            "cst": cstc,
        })

    import os
    trace = bool(os.environ.get("KERNEL_TRACE"))
    res = run_bass_kernel_spmd(_prog, in_maps, core_ids=list(range(NCORES)),
                               trace=trace)
    last_exec_ns = res.exec_time_ns
    out = np.empty((B, S, D), dtype=np.float32)
    for c in range(NCORES):
        b, hh = c // 2, c % 2
        out[b, hh * TQ:(hh + 1) * TQ, :] = res.results[c]["yT"].T
    return out
